# revision 16
# baseline (speedup 1.0000x reference)
"""TRN2 Bass/Tile kernel for nn_Attention (B=4, H=16, S=2048, D=64, fp32).

Entry point: kernel(q, k, v) -> out, all full-shape [4, 16, 2048, 64] fp32.

Sharding: batch*heads = 64 head-slices, 8 per NeuronCore (data/head
parallel, no cross-core communication). Each core runs the same NEFF on
its own 8 slices via run_bass_kernel_spmd.

v2 design (vs the f32r baseline at 379 us):
  - Q/K are converted to bf16 on DVE, staged to DRAM scratch in a packed
    [rows, 128] layout, and transposed by the DMA xbar engine
    (dma_start_transpose) straight into the D-major SBUF layouts the PE
    needs (qT duplicated across both partition halves, kT in the
    even/odd pair layout).  This removes all Q/K PE transposes and the
    per-chunk DVE PSUM->SBUF copies of the baseline.
  - QK^T runs in bf16 (same 1 cycle/row PE rate as f32r): two row-packed
    K=64 matmuls per chunk pair at tile_position (0,0)/(64,0).
  - softmax exp (33.5M elements/core -- the baseline bottleneck: ACT does
    1 elem/lane/cycle @ 1.2 GHz = 293 us/core alone) is split between
    ACT (exact spline exp) and DVE (Schraudolph bit-trick:
    one fused tensor_scalar mult+add writing int32 bit patterns through
    a bitcast AP into an f32r tile; ~1.8% multiplicative noise, mean
    calibrated to 1.0 so ACT/DVE chunks mix without bias).
  - PV accumulates O_aug^T[65, 512] over j-chunks in f32r with
    Vtilde = [V | ones]; row 64 = softmax denominator.
  - Epilogue: ACT copies PSUM->SBUF, PE transposes back, DVE applies the
    reciprocal row-scale, DMA out.

This container's walrus build rejects sync waits on Drain instructions
and allows at most one sync wait on any other instruction, while Tile
freely attaches several; _patch_tile_framework() + _split_sync_waits()
below rework the exit barrier and hoist excess waits onto injected NOPs.
"""
import sys

if '/opt/trn_rl_repo' not in sys.path:
    sys.path.insert(0, '/opt/trn_rl_repo')

import numpy as np

import concourse.bass as bass
import concourse.tile as tile
from concourse import mybir
from concourse.masks import make_identity
from concourse.vector_clock import ScopedClock

F32 = mybir.dt.float32
FP8 = mybir.dt.float8e4
F32R = mybir.dt.float32r
BF16 = mybir.dt.bfloat16
I16 = mybir.dt.int16
EXP = mybir.ActivationFunctionType.Exp

B, H, S, D = 4, 16, 2048, 64
N_CORES = 8
HEADS_PER_CORE = B * H // N_CORES

LOG2E = 1.4426950408889634
EXP_A = 0.125 * LOG2E * (1 << 7)           # scale 1/sqrt(D) folded in
EXP_B = (127.0 - 0.0579) * (1 << 7)        # mean-1.0 Schraudolph magic (bf16)


SKIP_EXP = False
SKIP_PV = False
ONLY_LOAD = False
USE_DR = False


def _dve_chunk(g, cc):
    """Which (i-group, chunk-pair) exp instructions go to DVE vs ACT."""
    return cc in (1, 3, 5) or (cc == 7 and g % 2 == 0)


# ---------------------------------------------------------------------------
# Walrus compatibility patches
# ---------------------------------------------------------------------------
_patched = False
_split_counter = [0]


def _patched_multi_engine_barrier(self, engines):
    for e in engines:
        self.engines[e].drain(fusable=False)
    for inst in self._sem_only_all_engine_barrier_insts(f"aeb{self.next_id()}"):
        self.engines[inst.engine].add_instruction(inst)


def _patched_drain_and_barrier(self, tick_clock, wait_clock):
    nop_inst = self.nc.sync.nop(nofuse=True, hint="tile_exit_wait")
    wait_clock.add_sem_waits(
        nop_inst.ins, ScopedClock({None: tick_clock.global_clock})
    )
    self.nc.sync.drain()
    self.nc.all_engine_barrier()
    assert self.sems is not None
    popped = self.nc._tile_sem_poison_stack.pop()
    assert popped is self._sem_poison
    self.nc.clear_and_free_semaphores(list(self.sems.allocated().values()))
    self.nc.all_engine_barrier()


def _patch_tile_framework():
    global _patched
    if _patched:
        return
    bass.Bass.multi_engine_barrier = _patched_multi_engine_barrier
    tile.TileContext._drain_and_barrier = _patched_drain_and_barrier
    _patched = True


def _split_sync_waits(nc):
    """No instruction may carry more than the walrus-supported number of
    sync waits (0 for Drain, 1 otherwise); hoist the rest onto NOPs."""
    for f in nc.m.functions:
        for bb in f.blocks:
            insts = bb.instructions
            if not any(
                i.sync_info is not None
                and len(i.sync_info.on_wait) > (0 if i.opcode == "Drain" else 1)
                for i in insts
            ):
                continue
            out = []
            for inst in insts:
                si = inst.sync_info
                limit = 0 if inst.opcode == "Drain" else 1
                if si is not None and len(si.on_wait) > limit:
                    waits = list(si.on_wait)
                    keep, extra = waits[:limit], waits[limit:]
                    for w in extra:
                        _split_counter[0] += 1
                        nop = mybir.InstNoOp(
                            name=f"waitsplit-{_split_counter[0]}", ins=[], outs=[]
                        )
                        nop.engine = inst.engine
                        nop.sync_info = mybir.SyncInfo(on_wait=[w], on_update=[])
                        out.append(nop)
                    inst.sync_info = mybir.SyncInfo(
                        on_wait=keep, on_update=list(si.on_update)
                    )
                out.append(inst)
            bb.instructions = out


# ---------------------------------------------------------------------------
# Kernel builder
# ---------------------------------------------------------------------------
def build_nc(heads=HEADS_PER_CORE, s=S, reps=1):
    NJ = s // 128           # j (k-row) chunks of 128
    IG = 512                # i (q-row) group width
    NG = s // IG
    NT = IG // 128
    scale = D ** -0.5

    nc = bass.Bass(target_bir_lowering=False)
    q_d = nc.dram_tensor("q", [heads, s, D], F32, kind="ExternalInput")
    k_d = nc.dram_tensor("k", [heads, s, D], F32, kind="ExternalInput")
    v_d = nc.dram_tensor("v", [heads, s, D], F32, kind="ExternalInput")
    o_d = nc.dram_tensor("o", [heads, s, D], F32, kind="ExternalOutput")

    with tile.TileContext(nc) as tc:
        with (
            tc.tile_pool(name="singles", bufs=1) as singles,
            tc.tile_pool(name="qkin", bufs=2) as qkin,
            tc.tile_pool(name="qkb", bufs=2) as qkb,
            tc.tile_pool(name="qkT", bufs=2) as qkT,
            tc.tile_pool(name="exps", bufs=3) as exps,
            tc.tile_pool(name="exps8", bufs=5) as exps8,
            tc.tile_pool(name="osb", bufs=2) as osb,
            tc.tile_pool(name="scr", bufs=2, space="DRAM") as scr,
            tc.tile_pool(name="qkps", bufs=3, space="PSUM") as qkps,
            tc.tile_pool(name="pvps", bufs=1, space="PSUM") as pvps,
            tc.tile_pool(name="trep", bufs=1, space="PSUM") as trep,
        ):
            ident = singles.tile([128, 128], F32)
            make_identity(nc, ident)
            identb = singles.tile([128, 128], BF16)
            make_identity(nc, identb)

            def head_attention(h, e, qT2, kT2, vl, vl8):
                """Attention for head h using partition half e of the
                transposed layouts; software-pipelined so the PE stream
                runs QK(t) ahead of PV(t-1)."""
                NP = NJ // 2
                P0 = 64 * e
                pairs = [(g, cc) for g in range(NG) for cc in range(NP)]
                pvs = {}

                def emit_epilogue(g):
                    og = osb.tile([D + 1, IG], BF16, tag="og")
                    nc.vector.tensor_copy(og, pvs[g])
                    oo = osb.tile([128, NT, D], F32, tag="oo")
                    for t in range(NT):
                        tr = trep.tile([128, D + 1], BF16, tag="tr")
                        nc.tensor.transpose(
                            tr, og[:, t * 128:(t + 1) * 128],
                            identb[0:D + 1, 0:D + 1])
                        rc = osb.tile([128, 1], F32, tag="rc")
                        nc.vector.reciprocal(rc, tr[:, D:D + 1])
                        nc.vector.tensor_scalar_mul(
                            oo[:, t, :], tr[:, 0:D], rc)
                    nc.sync.dma_start(
                        out=o_d[h, g * IG:(g + 1) * IG, :].rearrange(
                            "(t p) d -> p t d", p=128),
                        in_=oo)

                def emit_pv(g, cc, et, dve):
                    if cc == 0:
                        pvs[g] = pvps.tile(
                            [D + 1, IG], F32, tag="pv", name=f"pv{g}")
                    if dve or not USE_DR:
                        for half in range(2):
                            c = 2 * cc + half
                            nc.tensor.matmul(
                                pvs[g],
                                vl[:, e, c, :],
                                et[:, half * IG:(half + 1) * IG],
                                start=(c == 0), stop=(c == NJ - 1))
                    else:
                        nc.tensor.matmul(
                            pvs[g],
                            vl8[:, e, 2 * cc:2 * cc + 2, 0:D + 1],
                            et.rearrange("p (u i) -> p u i", u=2),
                            start=(cc == 0), stop=(cc == NP - 1),
                            perf_mode=mybir.MatmulPerfMode.DoubleRow)
                    if cc == NP - 1:
                        emit_epilogue(g)

                pend = None
                for g, cc in pairs:
                    dve = _dve_chunk(g, cc) and not SKIP_EXP
                    ps = qkps.tile([128, 2 * IG], F32, tag="ps")
                    for half in range(2):
                        c = 2 * cc + half
                        nc.tensor.matmul(
                            ps[:, half * IG:(half + 1) * IG],
                            kT2[P0:P0 + 64, c * 128:(c + 1) * 128],
                            qT2[P0:P0 + 64, g * IG:(g + 1) * IG],
                            start=True, stop=True,
                            tile_position=(P0, 0))
                    if SKIP_EXP:
                        et = exps.tile([128, 2 * IG], BF16, tag="et")
                        nc.vector.tensor_scalar(
                            et[:, 0:8].bitcast(I16), ps[:, 0:8],
                            EXP_A, EXP_B,
                            op0=mybir.AluOpType.mult,
                            op1=mybir.AluOpType.add)
                    elif dve:
                        et = exps.tile([128, 2 * IG], BF16, tag="et")
                        nc.vector.tensor_scalar(
                            et[:, :].bitcast(I16), ps, EXP_A, EXP_B,
                            op0=mybir.AluOpType.mult,
                            op1=mybir.AluOpType.add)
                    elif USE_DR:
                        et = exps8.tile([128, 2 * IG], FP8, tag="et8")
                        nc.scalar.activation(et, ps, EXP, scale=scale)
                    else:
                        et = exps.tile([128, 2 * IG], BF16, tag="et")
                        nc.scalar.activation(et, ps, EXP, scale=scale)
                    if pend is not None and not SKIP_PV:
                        emit_pv(*pend)
                    pend = (g, cc, et, dve)
                if not SKIP_PV:
                    emit_pv(*pend)

            def body():
                for hp in range(heads // 2):
                    # ---- per head-pair: load fp32, convert on DVE, stage
                    # both heads side by side in [rows, 128] DRAM scratch,
                    # DMA-xbar-transpose into D-major SBUF layouts (head
                    # 2hp on partitions 0-63, head 2hp+1 on 64-127) ----
                    qn = qkin.tile([128, 2, NJ, D], F32, tag="qn")
                    kn = qkin.tile([128, 2, NJ, D], F32, tag="kn")
                    vn = qkin.tile([128, 2, NJ, D], F32, tag="vn")
                    for e in range(2):
                        h = 2 * hp + e
                        nc.sync.dma_start(
                            out=qn[:, e],
                            in_=q_d[h].rearrange("(c p) d -> p c d", p=128))
                        nc.sync.dma_start(
                            out=kn[:, e],
                            in_=k_d[h].rearrange("(c p) d -> p c d", p=128))
                        nc.sync.dma_start(
                            out=vn[:, e],
                            in_=v_d[h].rearrange("(c p) d -> p c d", p=128))
                    vl = qkin.tile([128, 2, NJ, D + 1], BF16, tag="vl")
                    nc.vector.tensor_copy(vl[:, :, :, 0:D], vn)
                    nc.vector.memset(vl[:, :, :, D:D + 1], 1.0)
                    vl8 = qkin.tile([128, 2, NJ, 80], FP8, tag="vl8")
                    nc.vector.memset(vl8, 0.0)
                    nc.vector.tensor_copy(vl8[:, :, :, 0:D], vn)
                    nc.vector.memset(vl8[:, :, :, D:D + 1], 1.0)
                    qb = qkb.tile([128, 2, NJ, D], BF16, tag="qb")
                    kb = qkb.tile([128, 2, NJ, D], BF16, tag="kb")
                    nc.vector.tensor_copy(qb, qn)
                    nc.vector.tensor_copy(kb, kn)

                    qsc = scr.tile([s, 128], BF16, tag="qsc")
                    ksc = scr.tile([s, 128], BF16, tag="ksc")
                    qsc_v = qsc.rearrange("(c p) (e d) -> p c e d", p=128, e=2)
                    ksc_v = ksc.rearrange("(c p) (e d) -> p c e d", p=128, e=2)
                    for e in range(2):
                        nc.sync.dma_start(out=qsc_v[:, :, e, :], in_=qb[:, e])
                        nc.sync.dma_start(out=ksc_v[:, :, e, :], in_=kb[:, e])

                    qT2 = qkT.tile([128, s], BF16, tag="qT2")
                    kT2 = qkT.tile([128, s], BF16, tag="kT2")
                    nc.sync.dma_start_transpose(qT2, qsc[:, :])
                    nc.sync.dma_start_transpose(kT2, ksc[:, :])

                    if ONLY_LOAD:
                        dummy = qkps.tile([128, 2 * IG], F32, tag="ps")
                        nc.tensor.matmul(
                            dummy[:, 0:IG], kT2[0:64, 0:128], qT2[0:64, 0:IG],
                            start=True, stop=True, tile_position=(0, 0))
                        dm2 = exps.tile([128, 8], BF16, tag="et")
                        nc.vector.tensor_scalar(
                            dm2[:, :].bitcast(I16), dummy[:, 0:8], EXP_A, EXP_B,
                            op0=mybir.AluOpType.mult, op1=mybir.AluOpType.add)
                        nc.vector.tensor_copy(
                            dm2[:, 0:1].bitcast(I16),
                            vl[:, 0, 0, 0:1].bitcast(I16))
                        nc.vector.tensor_copy(
                            dm2[:, 1:2].bitcast(I16),
                            vl8[:, 0, 0, 0:2].bitcast(I16))
                        continue

                    for e in range(2):
                        head_attention(2 * hp + e, e, qT2, kT2, vl, vl8)

            if reps == 1:
                body()
            else:
                with tc.For_i(0, reps, 1):
                    body()

    _split_sync_waits(nc)
    return nc


_cached_nc = None


def _get_nc():
    global _cached_nc
    if _cached_nc is None:
        _patch_tile_framework()
        _cached_nc = build_nc()
    return _cached_nc


def kernel(q, k, v):
    """Full-shape attention: q/k/v [4, 16, 2048, 64] fp32 -> same shape."""
    from concourse.bass_utils import run_bass_kernel_spmd

    nc = _get_nc()
    q = np.ascontiguousarray(np.asarray(q, dtype=np.float32)).reshape(B * H, S, D)
    k = np.ascontiguousarray(np.asarray(k, dtype=np.float32)).reshape(B * H, S, D)
    v = np.ascontiguousarray(np.asarray(v, dtype=np.float32)).reshape(B * H, S, D)
    hpc = HEADS_PER_CORE
    in_maps = [
        {"q": q[i * hpc:(i + 1) * hpc],
         "k": k[i * hpc:(i + 1) * hpc],
         "v": v[i * hpc:(i + 1) * hpc]}
        for i in range(N_CORES)
    ]
    res = run_bass_kernel_spmd(nc, in_maps, core_ids=list(range(N_CORES)))
    out = np.concatenate([res.results[i]["o"] for i in range(N_CORES)], axis=0)
    return out.reshape(B, H, S, D)


# revision 18
# speedup vs baseline: 1.3051x; 1.3051x over previous
"""TRN2 Bass/Tile kernel for nn_Attention (B=4, H=16, S=2048, D=64, fp32).

Entry point: kernel(q, k, v) -> out, all full-shape [4, 16, 2048, 64] fp32.

Sharding: batch*heads = 64 head-slices, 8 per NeuronCore (data/head
parallel, no cross-core communication). Each core runs the same NEFF on
its own 8 slices via run_bass_kernel_spmd.

Design (vs the f32r baseline at 379 us; this version measures ~305 us):
  - Q/K are converted to bf16 on DVE, staged to DRAM scratch in a packed
    [rows, 128] layout (Q duplicated into both 64-col halves, K in an
    even/odd chunk pair layout), and transposed by the DMA xbar engine
    (dma_start_transpose) straight into the D-major SBUF layouts the PE
    needs.  This removes all Q/K PE transposes and the per-chunk DVE
    PSUM->SBUF copies of the baseline, freeing DVE for exp work.
  - QK^T runs in bf16 (same 1 cycle/row PE rate as f32r): two K=64
    matmuls per chunk pair at tile_position (0,0)/(64,0).
  - softmax exp (33.5M elements/core -- the baseline bottleneck: ACT does
    1 elem/lane/cycle @ 1.2 GHz = 293 us/core alone) is split 18/14
    between ACT (exact spline exp, bf16 out) and DVE (Schraudolph exp
    bit-trick: one fused tensor_scalar mult+add writing int16 bf16 bit
    patterns through a bitcast AP; +-1.8% multiplicative noise, mean
    calibrated to 1.0 on-device so ACT/DVE chunks mix without bias).
  - The 32 (i-group, chunk-pair) tiles are software-pipelined: the PE
    instruction stream runs QK(t) ahead of PV(t-1) (qkps triple-
    buffered) so the PE never stalls on the exp of the tile it just
    produced.
  - PV accumulates O_aug^T[65, 512] over j-chunks in bf16 with
    Vtilde = [V | ones]; row 64 = softmax denominator.
  - Epilogue: ACT copies PSUM->SBUF, PE transposes back, DVE applies the
    reciprocal row-scale, DMA out.
  Measured on HW: every matmul is a self-loading LDWEIGHTS+stream with
  no cross-matmul overlap (walrus --enable-ldw-opt=false; forcing true
  crashes codegen), so the PE side (~1.07 us per chunk pair) is the
  bottleneck; exp is fully hidden behind it.  fp8 DoubleRow PV (probe-
  validated standalone) produced NaN in-kernel and is not used.

This container's walrus build rejects sync waits on Drain instructions
and allows at most one sync wait on any other instruction, while Tile
freely attaches several; _patch_tile_framework() + _split_sync_waits()
below rework the exit barrier and hoist excess waits onto injected NOPs.
"""
import sys

if '/opt/trn_rl_repo' not in sys.path:
    sys.path.insert(0, '/opt/trn_rl_repo')

import numpy as np

import concourse.bass as bass
import concourse.tile as tile
from concourse import mybir
from concourse.masks import make_identity
from concourse.vector_clock import ScopedClock

F32 = mybir.dt.float32
FP8 = mybir.dt.float8e4
F32R = mybir.dt.float32r
BF16 = mybir.dt.bfloat16
I16 = mybir.dt.int16
EXP = mybir.ActivationFunctionType.Exp

B, H, S, D = 4, 16, 2048, 64
N_CORES = 8
HEADS_PER_CORE = B * H // N_CORES

LOG2E = 1.4426950408889634
EXP_A = 0.125 * LOG2E * (1 << 7)           # scale 1/sqrt(D) folded in
EXP_B = (127.0 - 0.0579) * (1 << 7)        # mean-1.0 Schraudolph magic (bf16)


SKIP_EXP = False
SKIP_PV = False
ONLY_LOAD = False
USE_DR = False


def _dve_chunk(g, cc):
    """Which (i-group, chunk-pair) exp instructions go to DVE vs ACT."""
    return cc in (1, 3, 5) or (cc == 7 and g % 2 == 0)


# ---------------------------------------------------------------------------
# Walrus compatibility patches
# ---------------------------------------------------------------------------
_patched = False
_split_counter = [0]


def _patched_multi_engine_barrier(self, engines):
    for e in engines:
        self.engines[e].drain(fusable=False)
    for inst in self._sem_only_all_engine_barrier_insts(f"aeb{self.next_id()}"):
        self.engines[inst.engine].add_instruction(inst)


def _patched_drain_and_barrier(self, tick_clock, wait_clock):
    nop_inst = self.nc.sync.nop(nofuse=True, hint="tile_exit_wait")
    wait_clock.add_sem_waits(
        nop_inst.ins, ScopedClock({None: tick_clock.global_clock})
    )
    self.nc.sync.drain()
    self.nc.all_engine_barrier()
    assert self.sems is not None
    popped = self.nc._tile_sem_poison_stack.pop()
    assert popped is self._sem_poison
    self.nc.clear_and_free_semaphores(list(self.sems.allocated().values()))
    self.nc.all_engine_barrier()


def _patch_tile_framework():
    global _patched
    if _patched:
        return
    bass.Bass.multi_engine_barrier = _patched_multi_engine_barrier
    tile.TileContext._drain_and_barrier = _patched_drain_and_barrier
    _patched = True


def _split_sync_waits(nc):
    """No instruction may carry more than the walrus-supported number of
    sync waits (0 for Drain, 1 otherwise); hoist the rest onto NOPs."""
    for f in nc.m.functions:
        for bb in f.blocks:
            insts = bb.instructions
            if not any(
                i.sync_info is not None
                and len(i.sync_info.on_wait) > (0 if i.opcode == "Drain" else 1)
                for i in insts
            ):
                continue
            out = []
            for inst in insts:
                si = inst.sync_info
                limit = 0 if inst.opcode == "Drain" else 1
                if si is not None and len(si.on_wait) > limit:
                    waits = list(si.on_wait)
                    keep, extra = waits[:limit], waits[limit:]
                    for w in extra:
                        _split_counter[0] += 1
                        nop = mybir.InstNoOp(
                            name=f"waitsplit-{_split_counter[0]}", ins=[], outs=[]
                        )
                        nop.engine = inst.engine
                        nop.sync_info = mybir.SyncInfo(on_wait=[w], on_update=[])
                        out.append(nop)
                    inst.sync_info = mybir.SyncInfo(
                        on_wait=keep, on_update=list(si.on_update)
                    )
                out.append(inst)
            bb.instructions = out


# ---------------------------------------------------------------------------
# Kernel builder
# ---------------------------------------------------------------------------
def build_nc(heads=HEADS_PER_CORE, s=S, reps=1):
    NJ = s // 128           # j (k-row) chunks of 128
    IG = 512                # i (q-row) group width
    NG = s // IG
    NT = IG // 128
    scale = D ** -0.5

    nc = bass.Bass(target_bir_lowering=False)
    q_d = nc.dram_tensor("q", [heads, s, D], F32, kind="ExternalInput")
    k_d = nc.dram_tensor("k", [heads, s, D], F32, kind="ExternalInput")
    v_d = nc.dram_tensor("v", [heads, s, D], F32, kind="ExternalInput")
    o_d = nc.dram_tensor("o", [heads, s, D], F32, kind="ExternalOutput")

    with tile.TileContext(nc) as tc:
        with (
            tc.tile_pool(name="singles", bufs=1) as singles,
            tc.tile_pool(name="qkin", bufs=2) as qkin,
            tc.tile_pool(name="qkb", bufs=2) as qkb,
            tc.tile_pool(name="qkT", bufs=2) as qkT,
            tc.tile_pool(name="exps", bufs=6) as exps,
            tc.tile_pool(name="osb", bufs=2) as osb,
            tc.tile_pool(name="scr", bufs=2, space="DRAM") as scr,
            tc.tile_pool(name="qkps", bufs=3, space="PSUM") as qkps,
            tc.tile_pool(name="pvps", bufs=1, space="PSUM") as pvps,
            tc.tile_pool(name="trep", bufs=1, space="PSUM") as trep,
        ):
            ident = singles.tile([128, 128], F32)
            make_identity(nc, ident)
            identb = singles.tile([128, 128], BF16)
            make_identity(nc, identb)

            def body():
                for h in range(heads):
                    # ---- load fp32, convert to bf16 on DVE, stage packed
                    # DRAM scratch, let the DMA xbar transpose into the
                    # D-major SBUF layouts the PE needs ----
                    qn = qkin.tile([128, NJ, D], F32, tag="qn")
                    kn = qkin.tile([128, NJ, D], F32, tag="kn")
                    vn = qkin.tile([128, NJ, D], F32, tag="vn")
                    nc.sync.dma_start(
                        out=qn, in_=q_d[h].rearrange("(c p) d -> p c d", p=128))
                    nc.sync.dma_start(
                        out=kn, in_=k_d[h].rearrange("(c p) d -> p c d", p=128))
                    nc.sync.dma_start(
                        out=vn, in_=v_d[h].rearrange("(c p) d -> p c d", p=128))
                    vl = qkin.tile([128, NJ, D + 1], BF16, tag="vl")
                    nc.vector.tensor_copy(vl[:, :, 0:D], vn)
                    nc.vector.memset(vl[:, :, D:D + 1], 1.0)
                    qb = qkb.tile([128, NJ, D], BF16, tag="qb")
                    kb = qkb.tile([128, NJ, D], BF16, tag="kb")
                    nc.vector.tensor_copy(qb, qn)
                    nc.vector.tensor_copy(kb, kn)

                    qsc = scr.tile([s, 128], BF16, tag="qsc")
                    ksc = scr.tile([s // 2, 128], BF16, tag="ksc")
                    qsc_v = qsc.rearrange("(c p) (u e) -> p c u e", p=128, u=2)
                    nc.sync.dma_start(out=qsc_v[:, :, 0, :], in_=qb)
                    nc.sync.dma_start(out=qsc_v[:, :, 1, :], in_=qb)
                    ksc_v = ksc.rearrange("(c j) (u e) -> j c u e", j=128, u=2)
                    kb_v = kb.rearrange("p (c u) e -> p c u e", u=2)
                    nc.sync.dma_start(out=ksc_v[:, :, 0, :], in_=kb_v[:, :, 0, :])
                    nc.sync.dma_start(out=ksc_v[:, :, 1, :], in_=kb_v[:, :, 1, :])

                    qT = qkT.tile([128, s], BF16, tag="qT")
                    kT = qkT.tile([128, s // 2], BF16, tag="kT")
                    nc.sync.dma_start_transpose(qT, qsc[:, :])
                    nc.sync.dma_start_transpose(kT, ksc[:, :])

                    # ---- attention, software-pipelined over the 32
                    # (i-group, chunk-pair) tiles: the PE stream runs
                    # QK(t) ahead of PV(t-1) so it never stalls on the
                    # exp of the pair it just produced ----
                    NP = NJ // 2
                    pairs = [(g, cc) for g in range(NG) for cc in range(NP)]
                    pvs = {}

                    def emit_epilogue(g):
                        og = osb.tile([D + 1, IG], F32, tag="og")
                        nc.scalar.copy(og, pvs[g])
                        oo = osb.tile([128, NT, D], F32, tag="oo")
                        for t in range(NT):
                            tr = trep.tile([128, D + 1], F32, tag="tr")
                            nc.tensor.transpose(
                                tr, og[:, t * 128:(t + 1) * 128],
                                ident[0:D + 1, 0:D + 1])
                            rc = osb.tile([128, 1], F32, tag="rc")
                            nc.vector.reciprocal(rc, tr[:, D:D + 1])
                            nc.vector.tensor_scalar_mul(
                                oo[:, t, :], tr[:, 0:D], rc)
                        nc.sync.dma_start(
                            out=o_d[h, g * IG:(g + 1) * IG, :].rearrange(
                                "(t p) d -> p t d", p=128),
                            in_=oo)

                    def emit_pv(g, cc, et):
                        if cc == 0:
                            pvs[g] = pvps.tile(
                                [D + 1, IG], F32, tag="pv", name=f"pv{g}")
                        for half in range(2):
                            c = 2 * cc + half
                            nc.tensor.matmul(
                                pvs[g],
                                vl[:, c, :],
                                et[:, half * IG:(half + 1) * IG],
                                start=(c == 0), stop=(c == NJ - 1))
                        if cc == NP - 1:
                            emit_epilogue(g)

                    pend = None
                    for g, cc in pairs:
                        ps = qkps.tile([128, 2 * IG], F32, tag="ps")
                        et = exps.tile([128, 2 * IG], BF16, tag="et")
                        for half in range(2):
                            nc.tensor.matmul(
                                ps[:, half * IG:(half + 1) * IG],
                                kT[half * 64:half * 64 + 64,
                                   cc * 128:(cc + 1) * 128],
                                qT[half * 64:half * 64 + 64,
                                   g * IG:(g + 1) * IG],
                                start=True, stop=True,
                                tile_position=(half * 64, 0))
                        if _dve_chunk(g, cc):
                            nc.vector.tensor_scalar(
                                et[:, :].bitcast(I16), ps, EXP_A, EXP_B,
                                op0=mybir.AluOpType.mult,
                                op1=mybir.AluOpType.add)
                        else:
                            nc.scalar.activation(et, ps, EXP, scale=scale)
                        if pend is not None:
                            emit_pv(*pend)
                        pend = (g, cc, et)
                    emit_pv(*pend)

            if reps == 1:
                body()
            else:
                with tc.For_i(0, reps, 1):
                    body()

    _split_sync_waits(nc)
    return nc


_cached_nc = None


def _get_nc():
    global _cached_nc
    if _cached_nc is None:
        _patch_tile_framework()
        _cached_nc = build_nc()
    return _cached_nc


def kernel(q, k, v):
    """Full-shape attention: q/k/v [4, 16, 2048, 64] fp32 -> same shape."""
    from concourse.bass_utils import run_bass_kernel_spmd

    nc = _get_nc()
    q = np.ascontiguousarray(np.asarray(q, dtype=np.float32)).reshape(B * H, S, D)
    k = np.ascontiguousarray(np.asarray(k, dtype=np.float32)).reshape(B * H, S, D)
    v = np.ascontiguousarray(np.asarray(v, dtype=np.float32)).reshape(B * H, S, D)
    hpc = HEADS_PER_CORE
    in_maps = [
        {"q": q[i * hpc:(i + 1) * hpc],
         "k": k[i * hpc:(i + 1) * hpc],
         "v": v[i * hpc:(i + 1) * hpc]}
        for i in range(N_CORES)
    ]
    res = run_bass_kernel_spmd(nc, in_maps, core_ids=list(range(N_CORES)))
    out = np.concatenate([res.results[i]["o"] for i in range(N_CORES)], axis=0)
    return out.reshape(B, H, S, D)
